# revision 7
# baseline (speedup 1.0000x reference)
"""Binarized BasicBlock (BNN) forward on 8 Trainium2 NeuronCores.

Reference computation (per reference.py):
    xb  = sign(x);  wb = sign(w)
    y1  = conv3x3(xb, wb1, pad=1)
    a1  = hardtanh(bn1(y1))          # only sign(a1) feeds conv2
    y2  = conv3x3(sign(a1), wb2, pad=1)
    out = hardtanh(bn2(y2) + x)

Strategy (v2):
  - Data parallel: batch N=64 -> 8 images per core; weights/BN replicated.
  - PE is at the fp8-DoubleRow silicon ceiling with direct conv (9 shifted
    matmuls), so most images use 1D Winograd F(2,3) along the width:
    for an output column pair (2c, 2c+1):
        V1 = d0-d2, V2 = d1+d2, V3 = d2-d1, V4 = d1-d3   (d = padded cols
        2c-1 .. 2c+2, values +-1 from sign())
        M_j = sum_{ci,dr} U_j[dr] * V_j[row+dr-1]        (PE, PSUM acc over dr)
        y(2c)   = M1 + M2 + M3                           (vector)
        y(2c+1) = M2 - M3 - M4
    with U1=w0, U2=(w0+w1+w2)/2, U3=(w0-w1+w2)/2, U4=w2 (exact in fp8e4m3:
    +-0.5, +-1, +-1.5).  12 matmul-rows per output pair vs 18 direct = 1.5x
    less PE work.  All arithmetic stays exact (dyadic coefficients, integer
    sums < 2^23), so sign() matches the reference bit-exactly.
  - ND images stay on the direct 9-tap path to balance PE vs vector engines
    (the Winograd transforms cost extra DVE/Pool elementwise passes).
  - Binarized operands fp8e4 (DoubleRow fp8 perf mode, fp32 PSUM).
  - BN folded: sign(bn1(y)) = Sign(y*s1 + t1); final = clip(y2*s2+t2 + x).
"""

import sys

try:
    import concourse  # noqa: F401
except ImportError:  # pragma: no cover
    sys.path.insert(0, "/opt/trn_rl_repo")

import numpy as np
import ml_dtypes

import concourse.bacc as bacc
import concourse.tile as tile
import concourse.mybir as mybir
from concourse.bass_utils import run_bass_kernel_spmd

dt = mybir.dt
AF = mybir.ActivationFunctionType
ALU = mybir.AluOpType
PM = mybir.MatmulPerfMode

N_CORES = 8
NPER = 8          # images per core
ND = 2            # images on the direct 9-tap path (PE/vector balance knob)
C = 256
H = W = 56
HW = H * W        # 3136
WP = 58           # padded row width (1 + 56 + 1)
ROWW = 64         # allocated width per (row, k-plane) block (16B aligned)
RPITCH = 2 * ROWW  # 128 = row pitch (both k-planes interleaved per row)
PROWS = 58        # padded rows
PLSZ = PROWS * RPITCH  # 7424 = padded image tile length
RPC = 8           # direct path: output rows per matmul chunk
CHU = RPC * W     # 448
NCH = H // RPC    # 7 chunks per image (direct)
NC2 = W // 2      # 28 column pairs (winograd)
RW = 14           # winograd: output rows per chunk
NWCH = H // RW    # 4 chunks per image (winograd)
MFREE = RW * NC2  # 392 = winograd matmul free dim
VKP = 4 * NC2     # 112 = v-tile per-(row,k) pitch: 4 j-planes x 28 cols
VRP = 2 * VKP     # 224 = v-tile row pitch
VLSZ = PROWS * VRP  # 12992 = v-tile length
BN_EPS = 1e-5

_CACHE = {}


def _zero_pads(nc, t):
    """Zero the padding cells of a [128, PLSZ] row-interleaved image tile.

    Layout: element (row r, k-plane k, col c) at r*RPITCH + k*ROWW + c;
    c=1..56 hold image cols 0..55, c=0 and c=57..63 are zero pads, rows
    0 and 57 are zero pad rows.  Runs on ScalarE (memzero) to keep the
    Pool engine free for the Winograd transforms."""
    v = t[:]
    nc.scalar.memzero(v[:, 0:RPITCH])                      # top pad row
    nc.scalar.memzero(v[:, 57 * RPITCH:PLSZ])              # bottom pad row
    # per-block right pads c=57..63 plus the following block's c=0
    cols = v[:, 57:57 + 57 * RPITCH].rearrange("p (r k c) -> p r k c", k=2, c=ROWW)
    nc.vector.memset(cols[:, :, :, 0:8], 0.0)


def _rview(t):
    # [128, PROWS, 2, ROWW]
    return t[:].rearrange("p (r k c) -> p r k c", k=2, c=ROWW)


def _build():
    nc = bacc.Bacc("TRN2", target_bir_lowering=False, debug=False)

    x_d = nc.dram_tensor("x", [NPER, C, H, W], dt.float32, kind="ExternalInput").ap()
    w1_d = nc.dram_tensor("w1b", [2, 128, 9, C], dt.float8e4, kind="ExternalInput").ap()
    w2_d = nc.dram_tensor("w2b", [2, 128, 9, C], dt.float8e4, kind="ExternalInput").ap()
    u1_d = nc.dram_tensor("u1b", [2, 128, 12, C], dt.float8e4, kind="ExternalInput").ap()
    u2_d = nc.dram_tensor("u2b", [2, 128, 12, C], dt.float8e4, kind="ExternalInput").ap()
    s1_d = nc.dram_tensor("s1", [2, 128], dt.float32, kind="ExternalInput").ap()
    t1_d = nc.dram_tensor("t1", [2, 128], dt.float32, kind="ExternalInput").ap()
    s2_d = nc.dram_tensor("s2", [2, 128], dt.float32, kind="ExternalInput").ap()
    t2_d = nc.dram_tensor("t2", [2, 128], dt.float32, kind="ExternalInput").ap()
    out_d = nc.dram_tensor("out", [NPER, C, H, W], dt.float32, kind="ExternalOutput").ap()

    with tile.TileContext(nc) as tc:
        with (
            tc.tile_pool(name="wp", bufs=1) as wp,
            tc.tile_pool(name="xin", bufs=4) as xinp,
            tc.tile_pool(name="xb", bufs=2) as xbp,
            tc.tile_pool(name="vt", bufs=2) as vtp,
            tc.tile_pool(name="ab", bufs=1) as abp,
            tc.tile_pool(name="tmp", bufs=2) as tmpp,
            tc.tile_pool(name="ost", bufs=2) as ostp,
            tc.tile_pool(name="ps", bufs=2, space="PSUM") as psp,
            nc.sbuf_tensor([128, 2 * CHU], dt.float8e4) as warm_in,
        ):
            # ---- PE warm-up: junk matmuls on scratch data so the PE clock
            # ramps before the first real matmul.
            nc.gpsimd.memset(warm_in[:], 0.0)
            wv = warm_in[:].rearrange("p (k c) -> p k c", k=2)
            warm_ps = psp.tile([128, CHU], dt.float32, tag="ma", name="warm_ps")
            for _ in range(27):
                nc.tensor.matmul(
                    warm_ps[:], wv[:, :, 0:128], wv[:],
                    start=True, stop=True, perf_mode=PM.DoubleRow,
                )

            # w1 (direct taps) loads first on the sync queue (needed by the
            # first matmul); everything else deferred to gpsimd queue.
            w_sb = []
            for tag in ("w1", "w2"):
                t = wp.tile([128, 2, 9, C], dt.float8e4, tag=tag, name=f"w_{tag}")
                w_sb.append(t)
            u_sb = []
            for tag in ("u1", "u2"):
                t = wp.tile([128, 2, 12, C], dt.float8e4, tag=tag, name=f"u_{tag}")
                u_sb.append(t)
            nc.sync.dma_start(w_sb[0][:], w1_d.rearrange("q p k c -> p q k c"))
            bn_sb = [
                wp.tile([128, 2], dt.float32, tag=tag, name=f"bn_{tag}")
                for tag in ("s1", "t1", "s2", "t2")
            ]
            s1_sb, t1_sb, s2_sb, t2_sb = bn_sb

            state = {}

            def emit_input(n):
                """DMA loads + binarize (+ V1 transform for winograd images)."""
                quarts = ((0, 8), (8, 16), (24, 16), (40, 16)) if n == 0 else ((0, H),)
                xin = []
                xb = xbp.tile([128, PLSZ], dt.float8e4, tag="xb", name="xb")
                _zero_pads(nc, xb)
                xbv = _rview(xb)
                for q in range(2):
                    xi = xinp.tile([128, HW], dt.float32, tag="xin", name="xin")
                    xin.append(xi)
                for r0, nr in quarts:
                    for q in range(2):
                        xi = xin[q]
                        dma_eng = nc.gpsimd if (n == 0 and q == 1) else nc.sync
                        dma_eng.dma_start(
                            xi[:, r0 * W:(r0 + nr) * W],
                            x_d[n, q * 128:(q + 1) * 128, r0:r0 + nr].rearrange(
                                "p h w -> p (h w)"),
                        )
                        nc.scalar.activation(
                            xbv[:, 1 + r0:1 + r0 + nr, q, 1:57],
                            xi[:, r0 * W:(r0 + nr) * W].rearrange(
                                "p (h w) -> p h w", w=W),
                            AF.Sign,
                        )

                if n == 0:
                    # deferred non-urgent loads
                    nc.gpsimd.dma_start(
                        w_sb[1][:], w2_d.rearrange("q p k c -> p q k c"))
                    for t, bd in zip(bn_sb, (s1_d, t1_d, s2_d, t2_d)):
                        nc.gpsimd.dma_start(t[:], bd.rearrange("q p -> p q"))
                    nc.gpsimd.dma_start(
                        u_sb[0][:], u1_d.rearrange("q p t c -> p q t c"))
                    nc.gpsimd.dma_start(
                        u_sb[1][:], u2_d.rearrange("q p t c -> p q t c"))

                st = {"xin": xin, "xb": xb}
                if n >= ND:
                    v1 = vtp.tile([128, PROWS, 2, 4, NC2], dt.float8e4,
                                  tag="v1", name="v1")
                    _v_pads(nc, v1)
                    _v_build(nc, v1, xb, rows=(0, H))
                    st["v1"] = v1
                state[n] = st

            def _v_build(nc, v, src, rows):
                """Winograd width-transform of a padded sign image into the
                4 j-planes of v (order: V1, V4, V2, V3), for padded rows
                rows[0]+1 .. rows[0]+rows[1].  src cols: img col j at col
                j+1; T0 = even cols, T1 = odd cols.
                  V1[c] = T0[c] - T0[c+1]   (plane 0)
                  V4[c] = T1[c] - T1[c+1]   (plane 1)
                  V2[c] = T1[c] + T0[c+1]   (plane 2)
                  V3[c] = T0[c+1] - T1[c]   (plane 3)
                Runs on the Pool engine (V1 image-level) by default; caller
                picks engine via the eng arg bound below."""
                r0, nr = rows
                # (row, k) flattened: src row pitch 128, k pitch 64
                sf = src[:].rearrange("p (rk c) -> p rk c", c=ROWW)
                sfv = sf[:, 2 * (1 + r0):2 * (1 + r0 + nr), :]
                # [p, rk, c2, two]
                s2v = sfv.rearrange("p rk (c2 two) -> p rk c2 two", two=2)
                vf = v[:].rearrange("p r k j c -> p (r k) j c")
                vfv = vf[:, 2 * (1 + r0):2 * (1 + r0 + nr), :, :]
                # paired planes 0,1: out[.., jp] = in0[.., two] - in1[.., two]
                nc.gpsimd.tensor_tensor(
                    vfv[:, :, 0:2, :].rearrange("p rk j c -> p rk c j"),
                    s2v[:, :, 0:NC2, :],
                    s2v[:, :, 1:NC2 + 1, :],
                    ALU.subtract,
                )
                # plane 2: T1[c] + T0[c+1]
                nc.gpsimd.tensor_tensor(
                    vfv[:, :, 2, :],
                    s2v[:, :, 0:NC2, 1],
                    s2v[:, :, 1:NC2 + 1, 0],
                    ALU.add,
                )
                # plane 3: T0[c+1] - T1[c]
                nc.vector.tensor_tensor(
                    vfv[:, :, 3, :],
                    s2v[:, :, 1:NC2 + 1, 0],
                    s2v[:, :, 0:NC2, 1],
                    ALU.subtract,
                )

            def _v_pads(nc, v):
                vflat = v[:].rearrange("p r k j c -> p (r k j c)")
                nc.scalar.memzero(vflat[:, 0:VRP])
                nc.scalar.memzero(vflat[:, 57 * VRP:VLSZ])

            # ---------------- direct path (baseline) ----------------
            def conv1_direct(n):
                st = state[n]
                ab = abp.tile([128, PLSZ], dt.float8e4, tag="ab", name="ab")
                _zero_pads(nc, ab)
                abv = _rview(ab)
                xbv = _rview(st["xb"])
                for co in range(2):
                    for s in range(NCH):
                        ps = psp.tile([128, CHU], dt.float32,
                                      tag=("ma", "mb", "mc", "md")[s % 4],
                                      name="ps")
                        for kk in range(9):
                            r0 = RPC * s + kk // 3
                            rhs = xbv[:, r0:r0 + RPC, :, kk % 3:kk % 3 + W].rearrange(
                                "p r k c -> p k r c")
                            nc.tensor.matmul(
                                ps[:],
                                w_sb[0][:, :, kk, co * 128:(co + 1) * 128],
                                rhs,
                                start=(kk == 0),
                                stop=(kk == 8),
                                perf_mode=PM.DoubleRow,
                            )
                        psv = ps[:].rearrange("p (r c) -> p r c", c=W)
                        nc.scalar.activation(
                            abv[:, 1 + RPC * s:1 + RPC * s + RPC, co, 1:57], psv,
                            AF.Sign,
                            bias=t1_sb[:, co:co + 1], scale=s1_sb[:, co:co + 1],
                        )
                st["ab"] = ab

            def conv2_direct(n):
                st = state[n]
                abv = _rview(st["ab"])
                xin = st["xin"]
                for co in range(2):
                    ost = ostp.tile([128, HW], dt.float32, tag="ost", name="ost")
                    ostv = ost[:].rearrange("p (h w) -> p h w", w=W)
                    xinv = xin[co][:].rearrange("p (h w) -> p h w", w=W)
                    for s in range(NCH):
                        ps = psp.tile([128, CHU], dt.float32,
                                      tag=("ma", "mb", "mc", "md")[s % 4],
                                      name="ps")
                        for kk in range(9):
                            r0 = RPC * s + kk // 3
                            rhs = abv[:, r0:r0 + RPC, :, kk % 3:kk % 3 + W].rearrange(
                                "p r k c -> p k r c")
                            nc.tensor.matmul(
                                ps[:],
                                w_sb[1][:, :, kk, co * 128:(co + 1) * 128],
                                rhs,
                                start=(kk == 0),
                                stop=(kk == 8),
                                perf_mode=PM.DoubleRow,
                            )
                        psv = ps[:].rearrange("p (r c) -> p r c", c=W)
                        tm = tmpp.tile([128, RPC * W], dt.float32, tag="tmpd",
                                       bufs=3, name="tm")
                        tmv = tm[:].rearrange("p (r c) -> p r c", c=W)
                        nc.scalar.activation(
                            tmv, psv, AF.Identity,
                            bias=t2_sb[:, co:co + 1], scale=s2_sb[:, co:co + 1],
                        )
                        ov = ostv[:, RPC * s:RPC * s + RPC, :]
                        nc.vector.tensor_tensor(
                            ov, tmv, xinv[:, RPC * s:RPC * s + RPC, :], ALU.add
                        )
                        nc.vector.tensor_scalar(ov, ov, 1.0, -1.0, ALU.min, ALU.max)
                        if s == 3:
                            nc.sync.dma_start(
                                out_d[n, co * 128:(co + 1) * 128, 0:32].rearrange(
                                    "p h w -> p (h w)"),
                                ost[:, 0:32 * W],
                            )
                        elif s >= 4:
                            r0o, r1o = 8 * s, 8 * s + 8
                            nc.sync.dma_start(
                                out_d[n, co * 128:(co + 1) * 128, r0o:r1o].rearrange(
                                    "p h w -> p (h w)"),
                                ost[:, r0o * W:r1o * W],
                            )

            # ---------------- winograd path ----------------
            def _wino_mms(v, u, co, s):
                """12 matmuls for one (chunk, co-half): returns M tiles
                [A(V1/w0), D(V4/w2), B(V2), C(V3)] accumulated over dr."""
                vv = v[:]
                ms = []
                for j, mtag in enumerate(("ma", "mb", "mc", "md")):
                    m = psp.tile([128, MFREE], dt.float32, tag=mtag, name="m")
                    for dr in range(3):
                        rhs = vv[:, RW * s + dr:RW * s + dr + RW, :, j, :].rearrange(
                            "p r k c -> p k r c")
                        nc.tensor.matmul(
                            m[:],
                            u[:, :, dr * 4 + j, co * 128:(co + 1) * 128],
                            rhs,
                            start=(dr == 0),
                            stop=(dr == 2),
                            perf_mode=PM.DoubleRow,
                        )
                    ms.append(m)
                return ms

            def _combine(ms, yi):
                """y_even = A+B+C -> yi[...,0]; y_odd = B-C-D -> yi[...,1].
                ms order: [A, D, B, C].  A TensorTensor may read at most ONE
                PSUM operand, so the shared term B is staged to SBUF first
                (ScalarE), then every TT pairs one SBUF and one PSUM input."""
                A, D, B, Cm = ms
                bs = tmpp.tile([128, MFREE], dt.float32, tag="bs", name="bs")
                nc.scalar.copy(bs[:], B[:])
                te = tmpp.tile([128, MFREE], dt.float32, tag="te", name="te")
                to = tmpp.tile([128, MFREE], dt.float32, tag="to", name="to")
                yiv = yi[:].rearrange("p (rc par) -> p rc par", par=2)
                nc.vector.tensor_tensor(te[:], bs[:], A[:], ALU.add)
                nc.vector.tensor_tensor(to[:], bs[:], Cm[:], ALU.subtract)
                nc.vector.tensor_tensor(yiv[:, :, 0], te[:], Cm[:], ALU.add)
                nc.vector.tensor_tensor(yiv[:, :, 1], to[:], D[:], ALU.subtract)

            def conv1_wino(n):
                st = state[n]
                v1 = st["v1"]
                ab = abp.tile([128, PLSZ], dt.float8e4, tag="ab", name="ab")
                _zero_pads(nc, ab)
                abf = _rview(ab)
                v2 = vtp.tile([128, PROWS, 2, 4, NC2], dt.float8e4, tag="v2",
                              bufs=1, name="v2")
                _v_pads(nc, v2)
                for s in range(NWCH):
                    for co in range(2):
                        ms = _wino_mms(v1, u_sb[0], co, s)
                        yi = tmpp.tile([128, RW * W], dt.float32, tag="yi",
                                       name="yi")
                        _combine(ms, yi)
                        nc.scalar.activation(
                            abf[:, 1 + RW * s:1 + RW * s + RW, co, 1:57],
                            yi[:].rearrange("p (r c) -> p r c", c=W),
                            AF.Sign,
                            bias=t1_sb[:, co:co + 1], scale=s1_sb[:, co:co + 1],
                        )
                    # width-transform the finished 14-row slab for conv2
                    _v_build(nc, v2, ab, rows=(RW * s, RW))
                st["ab"] = ab
                st["v2"] = v2

            def conv2_wino(n):
                st = state[n]
                v2 = st["v2"]
                xin = st["xin"]
                osts = []
                for co in range(2):
                    ost = ostp.tile([128, HW], dt.float32, tag="ost", name="ost")
                    osts.append(ost)
                for s in range(NWCH):
                    for co in range(2):
                        ms = _wino_mms(v2, u_sb[1], co, s)
                        yi = tmpp.tile([128, RW * W], dt.float32, tag="yi",
                                       name="yi")
                        _combine(ms, yi)
                        be = tmpp.tile([128, RW * W], dt.float32, tag="be",
                                       name="be")
                        nc.scalar.activation(
                            be[:], yi[:], AF.Identity,
                            bias=t2_sb[:, co:co + 1], scale=s2_sb[:, co:co + 1],
                        )
                        ost = osts[co]
                        ov = ost[:, RW * W * s:RW * W * (s + 1)]
                        nc.gpsimd.tensor_tensor(
                            ov, be[:], xin[co][:, RW * W * s:RW * W * (s + 1)],
                            ALU.add,
                        )
                        nc.vector.tensor_scalar(ov, ov, 1.0, -1.0, ALU.min, ALU.max)
                        if s % 2 == 1:
                            r0o = (s - 1) * RW
                            nc.sync.dma_start(
                                out_d[n, co * 128:(co + 1) * 128,
                                      r0o:r0o + 2 * RW].rearrange(
                                    "p h w -> p (h w)"),
                                ost[:, r0o * W:(r0o + 2 * RW) * W],
                            )

            # ---------------- main loop ----------------
            emit_input(0)
            for n in range(NPER):
                if n < ND:
                    conv1_direct(n)
                else:
                    conv1_wino(n)
                if n + 1 < NPER:
                    emit_input(n + 1)
                if n < ND:
                    conv2_direct(n)
                else:
                    conv2_wino(n)
                del state[n]

    nc.compile()
    return nc


def _get_nc():
    if "nc" not in _CACHE:
        _CACHE["nc"] = _build()
    return _CACHE["nc"]


def _prep_weights(w):
    # [co, cin, kh, kw] -> [cin_chunk 2, cin 128, tap 9, co 256], binarized fp8e4
    a = np.sign(w.astype(np.float32))
    a = a.transpose(1, 2, 3, 0).reshape(2, 128, 9, C)
    return np.ascontiguousarray(a.astype(ml_dtypes.float8_e4m3))


def _prep_wino_weights(w):
    """[co, cin, 3, 3] -> [cin_chunk 2, cin 128, t 12, co 256] fp8e4 where
    t = dr*4 + jidx, plane order [U1, U4, U2, U3]:
      U1 = w0, U4 = w2, U2 = (w0+w1+w2)/2, U3 = (w0-w1+w2)/2  (of sign(w))."""
    wb = np.sign(w.astype(np.float32))  # [co, ci, 3, 3]
    planes = np.empty((12, C, C), np.float32)  # [t, co, ci]
    for dr in range(3):
        w0 = wb[:, :, dr, 0]
        w1 = wb[:, :, dr, 1]
        w2 = wb[:, :, dr, 2]
        planes[dr * 4 + 0] = w0
        planes[dr * 4 + 1] = w2
        planes[dr * 4 + 2] = (w0 + w1 + w2) * 0.5
        planes[dr * 4 + 3] = (w0 - w1 + w2) * 0.5
    # -> [ci, t, co] -> [2, 128, 12, co]
    a = planes.transpose(2, 0, 1).reshape(2, 128, 12, C)
    return np.ascontiguousarray(a.astype(ml_dtypes.float8_e4m3))


def _fold_bn(g, b, m, v):
    s = (g.astype(np.float32) / np.sqrt(v.astype(np.float32) + BN_EPS)).astype(np.float32)
    t = (b.astype(np.float32) - m.astype(np.float32) * s).astype(np.float32)
    return (
        np.ascontiguousarray(s.reshape(2, 128)),
        np.ascontiguousarray(t.reshape(2, 128)),
    )


def _make_in_maps(x, w1, g1, b1, m1, v1, w2, g2, b2, m2, v2):
    w1b = _prep_weights(w1)
    w2b = _prep_weights(w2)
    u1b = _prep_wino_weights(w1)
    u2b = _prep_wino_weights(w2)
    s1, t1 = _fold_bn(g1, b1, m1, v1)
    s2, t2 = _fold_bn(g2, b2, m2, v2)
    x = np.ascontiguousarray(x.astype(np.float32, copy=False))
    in_maps = []
    for c in range(N_CORES):
        in_maps.append({
            "x": x[c * NPER:(c + 1) * NPER],
            "w1b": w1b, "w2b": w2b, "u1b": u1b, "u2b": u2b,
            "s1": s1, "t1": t1, "s2": s2, "t2": t2,
        })
    return in_maps


def kernel(x, w1, g1, b1, m1, v1, w2, g2, b2, m2, v2):
    nc = _get_nc()
    in_maps = _make_in_maps(x, w1, g1, b1, m1, v1, w2, g2, b2, m2, v2)
    res = run_bass_kernel_spmd(nc, in_maps, list(range(N_CORES)))
    out = np.concatenate([res.results[c]["out"] for c in range(N_CORES)], axis=0)
    return out


# revision 8
# speedup vs baseline: 1.6141x; 1.6141x over previous
"""Binarized BasicBlock (BNN) forward on 8 Trainium2 NeuronCores.

Reference computation (per reference.py):
    xb  = sign(x);  wb = sign(w)
    y1  = conv3x3(xb, wb1, pad=1)
    a1  = hardtanh(bn1(y1))          # only sign(a1) feeds conv2
    y2  = conv3x3(sign(a1), wb2, pad=1)
    out = hardtanh(bn2(y2) + x)

Strategy (v2):
  - Data parallel: batch N=64 -> 8 images per core; weights/BN replicated.
  - PE is at the fp8-DoubleRow silicon ceiling with direct conv (9 shifted
    matmuls), so most images use 1D Winograd F(2,3) along the width:
    for an output column pair (2c, 2c+1):
        V1 = d0-d2, V2 = d1+d2, V3 = d2-d1, V4 = d1-d3   (d = padded cols
        2c-1 .. 2c+2, values +-1 from sign())
        M_j = sum_{ci,dr} U_j[dr] * V_j[row+dr-1]        (PE, PSUM acc over dr)
        y(2c)   = M1 + M2 + M3                           (vector)
        y(2c+1) = M2 - M3 - M4
    with U1=w0, U2=(w0+w1+w2)/2, U3=(w0-w1+w2)/2, U4=w2 (exact in fp8e4m3:
    +-0.5, +-1, +-1.5).  12 matmul-rows per output pair vs 18 direct = 1.5x
    less PE work.  All arithmetic stays exact (dyadic coefficients, integer
    sums < 2^23), so sign() matches the reference bit-exactly.
  - ND images stay on the direct 9-tap path to balance PE vs vector engines
    (the Winograd transforms cost extra DVE/Pool elementwise passes).
  - Binarized operands fp8e4 (DoubleRow fp8 perf mode, fp32 PSUM).
  - BN folded: sign(bn1(y)) = Sign(y*s1 + t1); final = clip(y2*s2+t2 + x).
"""

import sys

try:
    import concourse  # noqa: F401
except ImportError:  # pragma: no cover
    sys.path.insert(0, "/opt/trn_rl_repo")

import numpy as np
import ml_dtypes

import concourse.bacc as bacc
import concourse.tile as tile
import concourse.mybir as mybir
from concourse.bass_utils import run_bass_kernel_spmd

dt = mybir.dt
AF = mybir.ActivationFunctionType
ALU = mybir.AluOpType
PM = mybir.MatmulPerfMode

N_CORES = 8
NPER = 8          # images per core
ND = 3            # images on the direct 9-tap path (PE/vector balance knob)
C = 256
H = W = 56
HW = H * W        # 3136
WP = 58           # padded row width (1 + 56 + 1)
ROWW = 64         # allocated width per (row, k-plane) block (16B aligned)
RPITCH = 2 * ROWW  # 128 = row pitch (both k-planes interleaved per row)
PROWS = 58        # padded rows
PLSZ = PROWS * RPITCH  # 7424 = padded image tile length
RPC = 8           # direct path: output rows per matmul chunk
CHU = RPC * W     # 448
NCH = H // RPC    # 7 chunks per image (direct)
NRP = H // 2      # 28 output row pairs (winograd, height direction)
RW = 14           # winograd: output rows per chunk
RPW = RW // 2     # 7 row pairs per chunk
NWCH = H // RW    # 4 chunks per image (winograd)
MFREE = RPW * W   # 392 = winograd matmul free dim
BN_EPS = 1e-5

_CACHE = {}


def _zero_pads(nc, t):
    """Zero the padding cells of a [128, PLSZ] row-interleaved image tile.

    Layout: element (row r, k-plane k, col c) at r*RPITCH + k*ROWW + c;
    c=1..56 hold image cols 0..55, c=0 and c=57..63 are zero pads, rows
    0 and 57 are zero pad rows.  Runs on ScalarE (memzero) to keep the
    Pool engine free for the Winograd transforms."""
    v = t[:]
    nc.scalar.memzero(v[:, 0:RPITCH])                      # top pad row
    nc.scalar.memzero(v[:, 57 * RPITCH:PLSZ])              # bottom pad row
    # per-block right pads c=57..63 plus the following block's c=0
    cols = v[:, 57:57 + 57 * RPITCH].rearrange("p (r k c) -> p r k c", k=2, c=ROWW)
    nc.vector.memset(cols[:, :, :, 0:8], 0.0)


def _rview(t):
    # [128, PROWS, 2, ROWW]
    return t[:].rearrange("p (r k c) -> p r k c", k=2, c=ROWW)


def _build():
    nc = bacc.Bacc("TRN2", target_bir_lowering=False, debug=False)

    x_d = nc.dram_tensor("x", [NPER, C, H, W], dt.float32, kind="ExternalInput").ap()
    w1_d = nc.dram_tensor("w1b", [2, 128, 9, C], dt.float8e4, kind="ExternalInput").ap()
    w2_d = nc.dram_tensor("w2b", [2, 128, 9, C], dt.float8e4, kind="ExternalInput").ap()
    u1_d = nc.dram_tensor("u1b", [2, 128, 12, C], dt.float8e4, kind="ExternalInput").ap()
    u2_d = nc.dram_tensor("u2b", [2, 128, 12, C], dt.float8e4, kind="ExternalInput").ap()
    s1_d = nc.dram_tensor("s1", [2, 128], dt.float32, kind="ExternalInput").ap()
    t1_d = nc.dram_tensor("t1", [2, 128], dt.float32, kind="ExternalInput").ap()
    s2_d = nc.dram_tensor("s2", [2, 128], dt.float32, kind="ExternalInput").ap()
    t2_d = nc.dram_tensor("t2", [2, 128], dt.float32, kind="ExternalInput").ap()
    out_d = nc.dram_tensor("out", [NPER, C, H, W], dt.float32, kind="ExternalOutput").ap()

    with tile.TileContext(nc) as tc:
        with (
            tc.tile_pool(name="wp", bufs=1) as wp,
            tc.tile_pool(name="xin", bufs=4) as xinp,
            tc.tile_pool(name="xb", bufs=2) as xbp,
            tc.tile_pool(name="vt", bufs=2) as vtp,
            tc.tile_pool(name="ab", bufs=1) as abp,
            tc.tile_pool(name="tmp", bufs=2) as tmpp,
            tc.tile_pool(name="ost", bufs=2) as ostp,
            tc.tile_pool(name="ps", bufs=2, space="PSUM") as psp,
            nc.sbuf_tensor([128, 2 * CHU], dt.float8e4) as warm_in,
        ):
            # ---- PE warm-up: junk matmuls on scratch data so the PE clock
            # ramps before the first real matmul.
            nc.gpsimd.memset(warm_in[:], 0.0)
            wv = warm_in[:].rearrange("p (k c) -> p k c", k=2)
            warm_ps = psp.tile([128, CHU], dt.float32, tag="ma", name="warm_ps")
            for _ in range(27):
                nc.tensor.matmul(
                    warm_ps[:], wv[:, :, 0:128], wv[:],
                    start=True, stop=True, perf_mode=PM.DoubleRow,
                )

            # w1 (direct taps) loads first on the sync queue (needed by the
            # first matmul); everything else deferred to gpsimd queue.
            w_sb = []
            for tag in ("w1", "w2"):
                t = wp.tile([128, 2, 9, C], dt.float8e4, tag=tag, name=f"w_{tag}")
                w_sb.append(t)
            u_sb = []
            for tag in ("u1", "u2"):
                t = wp.tile([128, 2, 12, C], dt.float8e4, tag=tag, name=f"u_{tag}")
                u_sb.append(t)
            nc.sync.dma_start(w_sb[0][:], w1_d.rearrange("q p k c -> p q k c"))
            bn_sb = [
                wp.tile([128, 2], dt.float32, tag=tag, name=f"bn_{tag}")
                for tag in ("s1", "t1", "s2", "t2")
            ]
            s1_sb, t1_sb, s2_sb, t2_sb = bn_sb

            state = {}

            def emit_input(n):
                """DMA loads + binarize (+ V1 transform for winograd images)."""
                quarts = ((0, 8), (8, 16), (24, 16), (40, 16)) if n == 0 else ((0, H),)
                xin = []
                xb = xbp.tile([128, PLSZ], dt.float8e4, tag="xb", name="xb")
                _zero_pads(nc, xb)
                xbv = _rview(xb)
                for q in range(2):
                    xi = xinp.tile([128, HW], dt.float32, tag="xin", name="xin")
                    xin.append(xi)
                for r0, nr in quarts:
                    for q in range(2):
                        xi = xin[q]
                        dma_eng = nc.gpsimd if (n == 0 and q == 1) else nc.sync
                        dma_eng.dma_start(
                            xi[:, r0 * W:(r0 + nr) * W],
                            x_d[n, q * 128:(q + 1) * 128, r0:r0 + nr].rearrange(
                                "p h w -> p (h w)"),
                        )
                        nc.scalar.activation(
                            xbv[:, 1 + r0:1 + r0 + nr, q, 1:57],
                            xi[:, r0 * W:(r0 + nr) * W].rearrange(
                                "p (h w) -> p h w", w=W),
                            AF.Sign,
                        )

                if n == 0:
                    # deferred non-urgent loads
                    nc.gpsimd.dma_start(
                        w_sb[1][:], w2_d.rearrange("q p k c -> p q k c"))
                    for t, bd in zip(bn_sb, (s1_d, t1_d, s2_d, t2_d)):
                        nc.gpsimd.dma_start(t[:], bd.rearrange("q p -> p q"))
                    nc.gpsimd.dma_start(
                        u_sb[0][:], u1_d.rearrange("q p t c -> p q t c"))
                    nc.gpsimd.dma_start(
                        u_sb[1][:], u2_d.rearrange("q p t c -> p q t c"))

                st = {"xin": xin, "xb": xb}
                if n >= ND:
                    v1 = vtp.tile([128, 4, NRP, 2, ROWW], dt.float8e4,
                                  tag="v1", name="v1")
                    _v_build(nc, v1, xb, 0, NRP)
                    st["v1"] = v1
                state[n] = st

            def _v_build(nc, v, src, r0, r1):
                """Winograd height-transform of a padded sign image into the
                4 j-planes of v (order: V1, V4, V2, V3), for row pairs
                r0 <= rp < r1.  src row r holds img row r-1; E'[i] = src row
                2i, O'[i] = src row 2i+1:
                  V1[rp] = E'[rp] - E'[rp+1]   (plane 0)
                  V4[rp] = O'[rp] - O'[rp+1]   (plane 1)
                  V2[rp] = O'[rp] + E'[rp+1]   (plane 2)
                  V3[rp] = E'[rp+1] - O'[rp]   (plane 3)
                All operands are whole 128B-contiguous (k,col) rows; col pads
                are inherited from src (zero rows stay zero)."""
                srp = src[:].rearrange("p (i par kc) -> p i par kc",
                                       par=2, kc=RPITCH)
                vv = v[:]
                # planes 0,1 in one op: par=0 -> V1, par=1 -> V4
                nc.vector.tensor_tensor(
                    vv[:, 0:2, r0:r1, :, :].rearrange("p j rp k c -> p rp j (k c)"),
                    srp[:, r0:r1, :, :],
                    srp[:, r0 + 1:r1 + 1, :, :],
                    ALU.subtract,
                )
                # plane 2: O'[rp] + E'[rp+1]
                nc.gpsimd.tensor_tensor(
                    vv[:, 2, r0:r1, :, :].rearrange("p rp k c -> p rp (k c)"),
                    srp[:, r0:r1, 1, :],
                    srp[:, r0 + 1:r1 + 1, 0, :],
                    ALU.add,
                )
                # plane 3: E'[rp+1] - O'[rp]
                nc.gpsimd.tensor_tensor(
                    vv[:, 3, r0:r1, :, :].rearrange("p rp k c -> p rp (k c)"),
                    srp[:, r0 + 1:r1 + 1, 0, :],
                    srp[:, r0:r1, 1, :],
                    ALU.subtract,
                )

            # ---------------- direct path (baseline) ----------------
            def conv1_direct(n):
                st = state[n]
                ab = abp.tile([128, PLSZ], dt.float8e4, tag="ab", name="ab")
                _zero_pads(nc, ab)
                abv = _rview(ab)
                xbv = _rview(st["xb"])
                for co in range(2):
                    for s in range(NCH):
                        ps = psp.tile([128, CHU], dt.float32,
                                      tag=("ma", "mb", "mc", "md")[s % 4],
                                      name="ps")
                        for kk in range(9):
                            r0 = RPC * s + kk // 3
                            rhs = xbv[:, r0:r0 + RPC, :, kk % 3:kk % 3 + W].rearrange(
                                "p r k c -> p k r c")
                            nc.tensor.matmul(
                                ps[:],
                                w_sb[0][:, :, kk, co * 128:(co + 1) * 128],
                                rhs,
                                start=(kk == 0),
                                stop=(kk == 8),
                                perf_mode=PM.DoubleRow,
                            )
                        psv = ps[:].rearrange("p (r c) -> p r c", c=W)
                        nc.scalar.activation(
                            abv[:, 1 + RPC * s:1 + RPC * s + RPC, co, 1:57], psv,
                            AF.Sign,
                            bias=t1_sb[:, co:co + 1], scale=s1_sb[:, co:co + 1],
                        )
                st["ab"] = ab

            def conv2_direct(n):
                st = state[n]
                abv = _rview(st["ab"])
                xin = st["xin"]
                for co in range(2):
                    ost = ostp.tile([128, HW], dt.float32, tag="ost", name="ost")
                    ostv = ost[:].rearrange("p (h w) -> p h w", w=W)
                    xinv = xin[co][:].rearrange("p (h w) -> p h w", w=W)
                    for s in range(NCH):
                        ps = psp.tile([128, CHU], dt.float32,
                                      tag=("ma", "mb", "mc", "md")[s % 4],
                                      name="ps")
                        for kk in range(9):
                            r0 = RPC * s + kk // 3
                            rhs = abv[:, r0:r0 + RPC, :, kk % 3:kk % 3 + W].rearrange(
                                "p r k c -> p k r c")
                            nc.tensor.matmul(
                                ps[:],
                                w_sb[1][:, :, kk, co * 128:(co + 1) * 128],
                                rhs,
                                start=(kk == 0),
                                stop=(kk == 8),
                                perf_mode=PM.DoubleRow,
                            )
                        psv = ps[:].rearrange("p (r c) -> p r c", c=W)
                        tm = tmpp.tile([128, RPC * W], dt.float32, tag="tmpd",
                                       bufs=3, name="tm")
                        tmv = tm[:].rearrange("p (r c) -> p r c", c=W)
                        nc.scalar.activation(
                            tmv, psv, AF.Identity,
                            bias=t2_sb[:, co:co + 1], scale=s2_sb[:, co:co + 1],
                        )
                        ov = ostv[:, RPC * s:RPC * s + RPC, :]
                        nc.vector.tensor_tensor(
                            ov, tmv, xinv[:, RPC * s:RPC * s + RPC, :], ALU.add
                        )
                        nc.vector.tensor_scalar(ov, ov, 1.0, -1.0, ALU.min, ALU.max)
                        if s == 3:
                            nc.sync.dma_start(
                                out_d[n, co * 128:(co + 1) * 128, 0:32].rearrange(
                                    "p h w -> p (h w)"),
                                ost[:, 0:32 * W],
                            )
                        elif s >= 4:
                            r0o, r1o = 8 * s, 8 * s + 8
                            nc.sync.dma_start(
                                out_d[n, co * 128:(co + 1) * 128, r0o:r1o].rearrange(
                                    "p h w -> p (h w)"),
                                ost[:, r0o * W:r1o * W],
                            )

            # ---------------- winograd path ----------------
            def _wino_mms(v, u, co, s):
                """12 matmuls for one (chunk, co-half): returns M tiles
                [A(V1), D(V4), B(V2), C(V3)] accumulated over the 3 width
                taps dc (cols dc..dc+55 of the padded-col v planes)."""
                vv = v[:]
                ms = []
                for j, mtag in enumerate(("ma", "mb", "mc", "md")):
                    m = psp.tile([128, MFREE], dt.float32, tag=mtag, name="m")
                    for dc in range(3):
                        rhs = vv[:, j, RPW * s:RPW * s + RPW, :,
                                 dc:dc + W].rearrange("p r k c -> p k r c")
                        nc.tensor.matmul(
                            m[:],
                            u[:, :, dc * 4 + j, co * 128:(co + 1) * 128],
                            rhs,
                            start=(dc == 0),
                            stop=(dc == 2),
                            perf_mode=PM.DoubleRow,
                        )
                    ms.append(m)
                return ms

            def _combine(ms, yi):
                """y_even = A+B+C -> yi[...,0]; y_odd = B-C-D -> yi[...,1].
                ms order: [A, D, B, C].  A TensorTensor may read at most ONE
                PSUM operand, so the shared term B is staged to SBUF first
                (ScalarE), then every TT pairs one SBUF and one PSUM input."""
                A, D, B, Cm = ms
                bs = tmpp.tile([128, MFREE], dt.float32, tag="bs", name="bs")
                nc.scalar.copy(bs[:], B[:])
                te = tmpp.tile([128, MFREE], dt.float32, tag="te", name="te")
                to = tmpp.tile([128, MFREE], dt.float32, tag="to", name="to")
                yiv = yi[:].rearrange("p (rp par c) -> p rp par c",
                                      par=2, c=W)
                tev = te[:].rearrange("p (r c) -> p r c", c=W)
                tov = to[:].rearrange("p (r c) -> p r c", c=W)
                cv = Cm[:].rearrange("p (r c) -> p r c", c=W)
                dv = D[:].rearrange("p (r c) -> p r c", c=W)
                nc.vector.tensor_tensor(te[:], bs[:], A[:], ALU.add)
                nc.vector.tensor_tensor(to[:], bs[:], Cm[:], ALU.subtract)
                nc.vector.tensor_tensor(yiv[:, :, 0, :], tev, cv, ALU.add)
                nc.vector.tensor_tensor(yiv[:, :, 1, :], tov, dv, ALU.subtract)

            def conv1_wino(n):
                st = state[n]
                v1 = st["v1"]
                ab = abp.tile([128, PLSZ], dt.float8e4, tag="ab", name="ab")
                _zero_pads(nc, ab)
                abf = _rview(ab)
                v2 = vtp.tile([128, 4, NRP, 2, ROWW], dt.float8e4, tag="v2",
                              bufs=1, name="v2")
                v2_ranges = ((0, 6), (6, 13), (13, 20), (20, 28))
                for s in range(NWCH):
                    for co in range(2):
                        ms = _wino_mms(v1, u_sb[0], co, s)
                        yi = tmpp.tile([128, RW * W], dt.float32, tag="yi",
                                       name="yi")
                        _combine(ms, yi)
                        nc.scalar.activation(
                            abf[:, 1 + RW * s:1 + RW * s + RW, co, 1:57],
                            yi[:].rearrange("p (r c) -> p r c", c=W),
                            AF.Sign,
                            bias=t1_sb[:, co:co + 1], scale=s1_sb[:, co:co + 1],
                        )
                    # height-transform the finished 14-row slab for conv2
                    _v_build(nc, v2, ab, *v2_ranges[s])
                st["ab"] = ab
                st["v2"] = v2

            def conv2_wino(n):
                st = state[n]
                v2 = st["v2"]
                xin = st["xin"]
                osts = []
                for co in range(2):
                    ost = ostp.tile([128, HW], dt.float32, tag="ost", name="ost")
                    osts.append(ost)
                for s in range(NWCH):
                    for co in range(2):
                        ms = _wino_mms(v2, u_sb[1], co, s)
                        yi = tmpp.tile([128, RW * W], dt.float32, tag="yi",
                                       name="yi")
                        _combine(ms, yi)
                        be = tmpp.tile([128, RW * W], dt.float32, tag="be",
                                       name="be")
                        nc.scalar.activation(
                            be[:], yi[:], AF.Identity,
                            bias=t2_sb[:, co:co + 1], scale=s2_sb[:, co:co + 1],
                        )
                        ost = osts[co]
                        ov = ost[:, RW * W * s:RW * W * (s + 1)]
                        nc.gpsimd.tensor_tensor(
                            ov, be[:], xin[co][:, RW * W * s:RW * W * (s + 1)],
                            ALU.add,
                        )
                        nc.vector.tensor_scalar(ov, ov, 1.0, -1.0, ALU.min, ALU.max)
                        if s % 2 == 1:
                            r0o = (s - 1) * RW
                            nc.sync.dma_start(
                                out_d[n, co * 128:(co + 1) * 128,
                                      r0o:r0o + 2 * RW].rearrange(
                                    "p h w -> p (h w)"),
                                ost[:, r0o * W:(r0o + 2 * RW) * W],
                            )

            # ---------------- main loop ----------------
            emit_input(0)
            for n in range(NPER):
                if n < ND:
                    conv1_direct(n)
                else:
                    conv1_wino(n)
                if n + 1 < NPER:
                    emit_input(n + 1)
                if n < ND:
                    conv2_direct(n)
                else:
                    conv2_wino(n)
                del state[n]

    nc.compile()
    return nc


def _get_nc():
    if "nc" not in _CACHE:
        _CACHE["nc"] = _build()
    return _CACHE["nc"]


def _prep_weights(w):
    # [co, cin, kh, kw] -> [cin_chunk 2, cin 128, tap 9, co 256], binarized fp8e4
    a = np.sign(w.astype(np.float32))
    a = a.transpose(1, 2, 3, 0).reshape(2, 128, 9, C)
    return np.ascontiguousarray(a.astype(ml_dtypes.float8_e4m3))


def _prep_wino_weights(w):
    """[co, cin, 3, 3] -> [cin_chunk 2, cin 128, t 12, co 256] fp8e4 where
    t = dc*4 + jidx, plane order [U1, U4, U2, U3], built from the HEIGHT
    taps: U1 = w[0,dc], U4 = w[2,dc], U2 = (w[0,dc]+w[1,dc]+w[2,dc])/2,
    U3 = (w[0,dc]-w[1,dc]+w[2,dc])/2  (of sign(w))."""
    wb = np.sign(w.astype(np.float32))  # [co, ci, 3, 3]
    planes = np.empty((12, C, C), np.float32)  # [t, co, ci]
    for dc in range(3):
        w0 = wb[:, :, 0, dc]
        w1 = wb[:, :, 1, dc]
        w2 = wb[:, :, 2, dc]
        planes[dc * 4 + 0] = w0
        planes[dc * 4 + 1] = w2
        planes[dc * 4 + 2] = (w0 + w1 + w2) * 0.5
        planes[dc * 4 + 3] = (w0 - w1 + w2) * 0.5
    # -> [ci, t, co] -> [2, 128, 12, co]
    a = planes.transpose(2, 0, 1).reshape(2, 128, 12, C)
    return np.ascontiguousarray(a.astype(ml_dtypes.float8_e4m3))


def _fold_bn(g, b, m, v):
    s = (g.astype(np.float32) / np.sqrt(v.astype(np.float32) + BN_EPS)).astype(np.float32)
    t = (b.astype(np.float32) - m.astype(np.float32) * s).astype(np.float32)
    return (
        np.ascontiguousarray(s.reshape(2, 128)),
        np.ascontiguousarray(t.reshape(2, 128)),
    )


def _make_in_maps(x, w1, g1, b1, m1, v1, w2, g2, b2, m2, v2):
    w1b = _prep_weights(w1)
    w2b = _prep_weights(w2)
    u1b = _prep_wino_weights(w1)
    u2b = _prep_wino_weights(w2)
    s1, t1 = _fold_bn(g1, b1, m1, v1)
    s2, t2 = _fold_bn(g2, b2, m2, v2)
    x = np.ascontiguousarray(x.astype(np.float32, copy=False))
    in_maps = []
    for c in range(N_CORES):
        in_maps.append({
            "x": x[c * NPER:(c + 1) * NPER],
            "w1b": w1b, "w2b": w2b, "u1b": u1b, "u2b": u2b,
            "s1": s1, "t1": t1, "s2": s2, "t2": t2,
        })
    return in_maps


def kernel(x, w1, g1, b1, m1, v1, w2, g2, b2, m2, v2):
    nc = _get_nc()
    in_maps = _make_in_maps(x, w1, g1, b1, m1, v1, w2, g2, b2, m2, v2)
    res = run_bass_kernel_spmd(nc, in_maps, list(range(N_CORES)))
    out = np.concatenate([res.results[c]["out"] for c in range(N_CORES)], axis=0)
    return out
